# revision 15
# baseline (speedup 1.0000x reference)
"""CGCoupler Trainium2 Bass kernel.

out[n, ro[k]] += x1[n, r1[k]] * x2[n, r2[k]] * cg[k]  for all k, rows n.

Because the CG index tables address contiguous channel runs, the whole op
decomposes into ~147 contiguous-slice FMAs per row:
    out[:, o:o+d] += c * x1[:, a:a+d] * x2[:, b:b+d]
with d in {32, 64}.  We lay batch rows on the 128 SBUF partitions and the
640-wide feature dim on the free axis, fold T=8 row-tiles into each DVE
instruction via multi-dim access patterns, and merge slice-ops with equal
coefficient and affine offset progressions into single instructions.

Data-parallel across 8 NeuronCores: each core processes 2048 rows.
"""
import numpy as np

N_CORES = 8
P_DIM = 128
T_FOLD = 8          # row-tiles folded per DVE instruction group
N_CHUNKS = 3        # product-pair buffer chunks (SBUF sizing)

_BUILD_CACHE = {}


# ----------------------------------------------------------------------------
# Planning: decompose index tables into merged slice-op instructions
# ----------------------------------------------------------------------------

def _extract_sliceops(cg, r1, r2, ro):
    M = len(cg)
    ops = []
    k = 0
    while k < M:
        j = k + 1
        while (j < M and r1[j] == r1[j-1] + 1 and r2[j] == r2[j-1] + 1
               and ro[j] == ro[j-1] + 1 and cg[j] == cg[k]):
            j += 1
        ops.append((int(r1[k]), int(r2[k]), int(ro[k]), j - k, float(cg[k])))
        k = j
    return ops


def _build_plan(cg, r1, r2, ro, out_dim, n_chunks=N_CHUNKS):
    """Products are TensorTensor (4D APs allowed); accums are
    TensorScalarPtr (TS/STT), which the HW verifier limits to <=3D access
    patterns (partition + 2 free dims).  One free dim is the T-fold, so an
    accum instruction covers multiple slice-ops only when they collapse
    into one contiguous run (out offsets AND product slots stepping by d).
    """
    ops = _extract_sliceops(cg, r1, r2, ro)

    pair_order, pair_idx = [], {}
    for (a, b, o, d, c) in ops:
        key = (a, b, d)
        if key not in pair_idx:
            pair_idx[key] = len(pair_order)
            pair_order.append(key)

    total_elems = sum(d for (_, _, d) in pair_order)
    target = total_elems / n_chunks
    chunks, slot, chunk_sizes = [], {}, []
    cur, cur_sz = [], 0
    for key in pair_order:
        d = key[2]
        if cur_sz + d > target * 1.02 and len(chunks) < n_chunks - 1 and cur:
            chunks.append(cur); chunk_sizes.append(cur_sz)
            cur, cur_sz = [], 0
        slot[key] = (len(chunks), cur_sz)
        cur.append(key)
        cur_sz += d
    chunks.append(cur); chunk_sizes.append(cur_sz)

    # products: merge runs with constant (da, db, dslot), equal d (4D TT)
    prod_instrs = [[] for _ in range(n_chunks)]
    for ci, ch in enumerate(chunks):
        i = 0
        while i < len(ch):
            a0, b0, d0 = ch[i]
            s0 = slot[ch[i]][1]
            j = i + 1
            da = db = ds = None
            while j < len(ch):
                a1, b1, d1 = ch[j]
                if d1 != d0:
                    break
                nda = a1 - ch[j-1][0]
                ndb = b1 - ch[j-1][1]
                nds = slot[ch[j]][1] - slot[ch[j-1]][1]
                if da is None:
                    da, db, ds = nda, ndb, nds
                elif (nda, ndb, nds) != (da, db, ds):
                    break
                j += 1
            n = j - i
            if n == 1:
                da = db = ds = 0
            prod_instrs[ci].append(dict(pslot=s0, a=a0, b=b0, d=d0,
                                        da=da, db=db, ds=ds, n=n))
            i = j

    # accumulations: chunk-major, wide-first; first full-touch is a write
    acc_raw = [[] for _ in range(n_chunks)]
    for (a, b, o, d, c) in ops:
        ci, off = slot[(a, b, d)]
        acc_raw[ci].append(dict(o=o, pslot=off, c=c, d=d))
    covered = np.zeros(out_dim, bool)
    needs_memset = False
    per_chunk = []
    for ci in range(n_chunks):
        qs = sorted(acc_raw[ci], key=lambda q: (-q['d'], q['c'], q['o'], q['pslot']))
        for q in qs:
            rng = slice(q['o'], q['o'] + q['d'])
            if not covered[rng].any():
                q['kind'] = 'TS'
            else:
                if not covered[rng].all():
                    needs_memset = True
                q['kind'] = 'STT'
            covered[rng] = True
        per_chunk.append(qs)
    if not covered.all():
        needs_memset = True

    # merge only naturally-contiguous runs (collapse to [T, n*d], 3D)
    acc_instrs = [[] for _ in range(n_chunks)]
    for ci in range(n_chunks):
        qs = sorted(per_chunk[ci],
                    key=lambda q: (q['kind'] != 'TS', -q['d'], q['c'], q['o'], q['pslot']))
        i = 0
        while i < len(qs):
            q0 = qs[i]
            j = i + 1
            while j < len(qs):
                q1, qp = qs[j], qs[j-1]
                if q1['kind'] != q0['kind'] or q1['d'] != q0['d'] or q1['c'] != q0['c']:
                    break
                if q1['o'] - qp['o'] != q0['d'] or q1['pslot'] - qp['pslot'] != q0['d']:
                    break
                j += 1
            n = j - i
            acc_instrs[ci].append(dict(kind=q0['kind'], o=q0['o'], pslot=q0['pslot'],
                                       c=q0['c'], d=q0['d'], n=n))
            i = j

    return dict(chunk_sizes=chunk_sizes, prod_instrs=prod_instrs,
                acc_instrs=acc_instrs, needs_memset=needs_memset)


# ----------------------------------------------------------------------------
# Bass program
# ----------------------------------------------------------------------------

def _build_bass(plan, rows_per_core, rep_dim, out_dim):
    import concourse.bass as bass
    import concourse.mybir as mybir
    from concourse.ap import AP
    from concourse.tile import TileContext
    import concourse.tile as _tile_mod
    from concourse.vector_clock import ScopedClock as _ScopedClock

    # The kernel-tail Drain instruction waits on every proc lane with
    # outstanding ticks, but its CTRL ISA struct only has room for a couple
    # of embedded sync-wait commands ("Too many sync wait commands" in
    # walrus codegen otherwise).  Split the global-clock wait across
    # several Drain instructions, two procs each (waits already observed by
    # the SP engine are elided by add_sem_waits).
    if not getattr(_tile_mod.TileContext, '_cg_drain_patched', False):
        _orig_dab = _tile_mod.TileContext._drain_and_barrier

        def _split_drain_and_barrier(self, tick_clock, wait_clock):
            gc = tick_clock.global_clock
            VC = type(gc)
            procs = []
            for p in range(27):
                t = gc.peek_next(p) - 1
                if t > 0:
                    procs.append((p, t))
            for i in range(0, len(procs), 1):
                pc = VC()
                for p, t in procs[i:i + 1]:
                    for _ in range(t):
                        pc.advance(p)
                d = self.nc.sync.drain()
                wait_clock.add_sem_waits(d.ins, _ScopedClock({None: pc}))
            self.nc.all_engine_barrier()
            popped = self.nc._tile_sem_poison_stack.pop()
            assert popped is self._sem_poison
            self.nc.clear_and_free_semaphores(list(self.sems.allocated().values()))
            self.nc.all_engine_barrier()

        _tile_mod.TileContext._drain_and_barrier = _split_drain_and_barrier
        _tile_mod.TileContext._cg_drain_patched = True

    f32 = mybir.dt.float32
    T = T_FOLD
    n_groups = rows_per_core // (P_DIM * T)
    assert rows_per_core == n_groups * P_DIM * T

    nc = bass.Bass("TRN2")
    x1d = nc.declare_dram_parameter("x1", [rows_per_core, rep_dim], f32, isOutput=False)
    x2d = nc.declare_dram_parameter("x2", [rows_per_core, rep_dim], f32, isOutput=False)
    outd = nc.declare_dram_parameter("out", [rows_per_core, out_dim], f32, isOutput=True)

    def ap_custom(tile, base, dims):
        a = tile[:]
        aplist = [list(a.ap[0])] + [[s, n] for (s, n) in dims]
        return AP(a.tensor, a.offset + base, aplist)

    with TileContext(nc) as tc:
        with (
            tc.tile_pool(name="io", bufs=2) as iop,
            tc.tile_pool(name="pp", bufs=2) as ppp,
        ):
            def dram_group_ap(dram, g, width):
                # [128p, T, width] view of rows [g*T*128, (g+1)*T*128):
                # row = g*T*128 + t*128 + p, iterated (p, t, f)
                a = dram[:]
                return AP(a.tensor, g * T * P_DIM * width,
                          [[width, P_DIM], [P_DIM * width, T], [1, width]])

            for g in range(n_groups):
                X1 = iop.tile([P_DIM, T * rep_dim], f32, tag="X1")
                X2 = iop.tile([P_DIM, T * rep_dim], f32, tag="X2")
                O = iop.tile([P_DIM, T * out_dim], f32, tag="O")
                # one big DMA per tensor: >=1MiB transfers split across all
                # 16 SDMA engines, and compute instructions then wait on at
                # most a couple of DMA semaphores (HW wait-slot limit).
                nc.gpsimd.dma_start(X1[:], dram_group_ap(x1d, g, rep_dim))
                nc.gpsimd.dma_start(X2[:], dram_group_ap(x2d, g, rep_dim))
                # wait absorbers: 4D-AP TensorTensor instructions cannot
                # carry embedded sync waits (S3S3D3 struct), so soak up the
                # DMA-complete waits with tiny 2D copies first.
                SCR = iop.tile([P_DIM, 4], f32, tag="SCR")
                nc.vector.tensor_copy(SCR[:, 0:2], X1[:, 0:2])
                nc.vector.tensor_copy(SCR[:, 2:4], X2[:, 0:2])
                if plan['needs_memset']:
                    nc.gpsimd.memset(O[:], 0.0)

                for ci, csz in enumerate(plan['chunk_sizes']):
                    P = ppp.tile([P_DIM, T * csz], f32, tag="P")
                    for pi in plan['prod_instrs'][ci]:
                        dims = [(csz, T), (pi['ds'], pi['n']), (1, pi['d'])]
                        nc.vector.tensor_tensor(
                            ap_custom(P, pi['pslot'], dims),
                            ap_custom(X1, pi['a'],
                                      [(rep_dim, T), (pi['da'], pi['n']), (1, pi['d'])]),
                            ap_custom(X2, pi['b'],
                                      [(rep_dim, T), (pi['db'], pi['n']), (1, pi['d'])]),
                            mybir.AluOpType.mult,
                        )
                    for qi in plan['acc_instrs'][ci]:
                        w = qi['n'] * qi['d']   # collapsed contiguous width
                        o_ap = ap_custom(O, qi['o'], [(out_dim, T), (1, w)])
                        p_ap = ap_custom(P, qi['pslot'], [(csz, T), (1, w)])
                        if qi['kind'] == 'TS':
                            nc.vector.tensor_scalar_mul(o_ap, p_ap, float(qi['c']))
                        else:
                            nc.vector.scalar_tensor_tensor(
                                out=o_ap, in0=p_ap, scalar=float(qi['c']),
                                in1=o_ap,
                                op0=mybir.AluOpType.mult,
                                op1=mybir.AluOpType.add,
                            )

                nc.sync.dma_start(dram_group_ap(outd, g, out_dim), O[:])
    return nc


# ----------------------------------------------------------------------------
# Entry point
# ----------------------------------------------------------------------------

def kernel(x1, x2, cg_tilde, repids_in1, repids_in2, repids_out, out_dim):
    from concourse.bass_utils import run_bass_kernel_spmd

    x1 = np.asarray(x1, dtype=np.float32)
    x2 = np.asarray(x2, dtype=np.float32)
    cg = np.asarray(cg_tilde, dtype=np.float32)
    r1 = np.asarray(repids_in1).astype(np.int64)
    r2 = np.asarray(repids_in2).astype(np.int64)
    ro = np.asarray(repids_out).astype(np.int64)
    out_dim = int(out_dim)

    n, rep_dim = x1.shape
    rows_per_core = n // N_CORES

    key = (rows_per_core, rep_dim, out_dim, cg.tobytes(), r1.tobytes(),
           r2.tobytes(), ro.tobytes())
    cache_key = hash(key)
    if cache_key not in _BUILD_CACHE:
        plan = _build_plan(cg, r1, r2, ro, out_dim)
        nc = _build_bass(plan, rows_per_core, rep_dim, out_dim)
        _BUILD_CACHE[cache_key] = nc
    nc = _BUILD_CACHE[cache_key]

    in_maps = [
        {"x1": x1[i*rows_per_core:(i+1)*rows_per_core],
         "x2": x2[i*rows_per_core:(i+1)*rows_per_core]}
        for i in range(N_CORES)
    ]
    res = run_bass_kernel_spmd(nc, in_maps, list(range(N_CORES)))
    out = np.concatenate([res.results[i]["out"] for i in range(N_CORES)], axis=0)
    return out


# revision 18
# speedup vs baseline: 789.6796x; 789.6796x over previous
"""CGCoupler Trainium2 Bass kernel.

out[n, ro[k]] += x1[n, r1[k]] * x2[n, r2[k]] * cg[k]  for all k, rows n.

Because the CG index tables address contiguous channel runs, the whole op
decomposes into ~147 contiguous-slice FMAs per row:
    out[:, o:o+d] += c * x1[:, a:a+d] * x2[:, b:b+d]
with d in {32, 64}.  We lay batch rows on the 128 SBUF partitions and the
640-wide feature dim on the free axis, fold T=8 row-tiles into each DVE
instruction via multi-dim access patterns, and merge slice-ops with equal
coefficient and affine offset progressions into single instructions.

Data-parallel across 8 NeuronCores: each core processes 2048 rows.
"""
import numpy as np

N_CORES = 8
P_DIM = 128
T_FOLD = 8          # row-tiles folded per DVE instruction group
N_CHUNKS = 3        # product-pair buffer chunks (SBUF sizing)

_BUILD_CACHE = {}


# ----------------------------------------------------------------------------
# Planning: decompose index tables into merged slice-op instructions
# ----------------------------------------------------------------------------

def _extract_sliceops(cg, r1, r2, ro):
    M = len(cg)
    ops = []
    k = 0
    while k < M:
        j = k + 1
        while (j < M and r1[j] == r1[j-1] + 1 and r2[j] == r2[j-1] + 1
               and ro[j] == ro[j-1] + 1 and cg[j] == cg[k]):
            j += 1
        ops.append((int(r1[k]), int(r2[k]), int(ro[k]), j - k, float(cg[k])))
        k = j
    return ops


def _build_plan(cg, r1, r2, ro, out_dim, n_chunks=N_CHUNKS):
    """Products are TensorTensor (4D APs allowed); accums are
    TensorScalarPtr (TS/STT), which the HW verifier limits to <=3D access
    patterns (partition + 2 free dims).  One free dim is the T-fold, so an
    accum instruction covers multiple slice-ops only when they collapse
    into one contiguous run (out offsets AND product slots stepping by d).
    """
    ops = _extract_sliceops(cg, r1, r2, ro)

    pair_order, pair_idx = [], {}
    for (a, b, o, d, c) in ops:
        key = (a, b, d)
        if key not in pair_idx:
            pair_idx[key] = len(pair_order)
            pair_order.append(key)

    total_elems = sum(d for (_, _, d) in pair_order)
    target = total_elems / n_chunks
    chunks, slot, chunk_sizes = [], {}, []
    cur, cur_sz = [], 0
    for key in pair_order:
        d = key[2]
        if cur_sz + d > target * 1.02 and len(chunks) < n_chunks - 1 and cur:
            chunks.append(cur); chunk_sizes.append(cur_sz)
            cur, cur_sz = [], 0
        slot[key] = (len(chunks), cur_sz)
        cur.append(key)
        cur_sz += d
    chunks.append(cur); chunk_sizes.append(cur_sz)

    # products: merge runs with constant (da, db, dslot), equal d (4D TT)
    prod_instrs = [[] for _ in range(n_chunks)]
    for ci, ch in enumerate(chunks):
        i = 0
        while i < len(ch):
            a0, b0, d0 = ch[i]
            s0 = slot[ch[i]][1]
            j = i + 1
            da = db = ds = None
            while j < len(ch):
                a1, b1, d1 = ch[j]
                if d1 != d0:
                    break
                nda = a1 - ch[j-1][0]
                ndb = b1 - ch[j-1][1]
                nds = slot[ch[j]][1] - slot[ch[j-1]][1]
                if da is None:
                    da, db, ds = nda, ndb, nds
                elif (nda, ndb, nds) != (da, db, ds):
                    break
                j += 1
            n = j - i
            if n == 1:
                da = db = ds = 0
            prod_instrs[ci].append(dict(pslot=s0, a=a0, b=b0, d=d0,
                                        da=da, db=db, ds=ds, n=n))
            i = j

    # accumulations: chunk-major, wide-first; first full-touch is a write
    acc_raw = [[] for _ in range(n_chunks)]
    for (a, b, o, d, c) in ops:
        ci, off = slot[(a, b, d)]
        acc_raw[ci].append(dict(o=o, pslot=off, c=c, d=d))
    covered = np.zeros(out_dim, bool)
    needs_memset = False
    per_chunk = []
    for ci in range(n_chunks):
        qs = sorted(acc_raw[ci], key=lambda q: (-q['d'], q['c'], q['o'], q['pslot']))
        for q in qs:
            rng = slice(q['o'], q['o'] + q['d'])
            if not covered[rng].any():
                q['kind'] = 'TS'
            else:
                if not covered[rng].all():
                    needs_memset = True
                q['kind'] = 'STT'
            covered[rng] = True
        per_chunk.append(qs)
    if not covered.all():
        needs_memset = True

    # merge only naturally-contiguous runs (collapse to [T, n*d], 3D)
    acc_instrs = [[] for _ in range(n_chunks)]
    for ci in range(n_chunks):
        qs = sorted(per_chunk[ci],
                    key=lambda q: (q['kind'] != 'TS', -q['d'], q['c'], q['o'], q['pslot']))
        i = 0
        while i < len(qs):
            q0 = qs[i]
            j = i + 1
            while j < len(qs):
                q1, qp = qs[j], qs[j-1]
                if q1['kind'] != q0['kind'] or q1['d'] != q0['d'] or q1['c'] != q0['c']:
                    break
                if q1['o'] - qp['o'] != q0['d'] or q1['pslot'] - qp['pslot'] != q0['d']:
                    break
                j += 1
            n = j - i
            acc_instrs[ci].append(dict(kind=q0['kind'], o=q0['o'], pslot=q0['pslot'],
                                       c=q0['c'], d=q0['d'], n=n))
            i = j

    return dict(chunk_sizes=chunk_sizes, prod_instrs=prod_instrs,
                acc_instrs=acc_instrs, needs_memset=needs_memset)


# ----------------------------------------------------------------------------
# Bass program
# ----------------------------------------------------------------------------

def _build_bass(plan, rows_per_core, rep_dim, out_dim, repeat=1, compute_repeat=1):
    import concourse.bass as bass
    import concourse.mybir as mybir
    from concourse.ap import AP
    from concourse.tile import TileContext
    import concourse.tile as _tile_mod
    from concourse.vector_clock import ScopedClock as _ScopedClock

    # The kernel-tail Drain instruction waits on every proc lane with
    # outstanding ticks, but its CTRL ISA struct only has room for a couple
    # of embedded sync-wait commands ("Too many sync wait commands" in
    # walrus codegen otherwise).  Split the global-clock wait across
    # several Drain instructions, two procs each (waits already observed by
    # the SP engine are elided by add_sem_waits).
    if not getattr(_tile_mod.TileContext, '_cg_drain_patched', False):
        _orig_dab = _tile_mod.TileContext._drain_and_barrier

        def _split_drain_and_barrier(self, tick_clock, wait_clock):
            gc = tick_clock.global_clock
            VC = type(gc)
            procs = []
            for p in range(27):
                t = gc.peek_next(p) - 1
                if t > 0:
                    procs.append((p, t))
            for i in range(0, len(procs), 1):
                pc = VC()
                for p, t in procs[i:i + 1]:
                    for _ in range(t):
                        pc.advance(p)
                d = self.nc.sync.drain()
                wait_clock.add_sem_waits(d.ins, _ScopedClock({None: pc}))
            self.nc.all_engine_barrier()
            popped = self.nc._tile_sem_poison_stack.pop()
            assert popped is self._sem_poison
            self.nc.clear_and_free_semaphores(list(self.sems.allocated().values()))
            self.nc.all_engine_barrier()

        _tile_mod.TileContext._drain_and_barrier = _split_drain_and_barrier
        _tile_mod.TileContext._cg_drain_patched = True

    f32 = mybir.dt.float32
    T = T_FOLD
    n_groups = rows_per_core // (P_DIM * T)
    assert rows_per_core == n_groups * P_DIM * T

    nc = bass.Bass("TRN2")
    x1d = nc.declare_dram_parameter("x1", [rows_per_core, rep_dim], f32, isOutput=False)
    x2d = nc.declare_dram_parameter("x2", [rows_per_core, rep_dim], f32, isOutput=False)
    outd = nc.declare_dram_parameter("out", [rows_per_core, out_dim], f32, isOutput=True)

    def ap_custom(tile, base, dims):
        a = tile[:]
        aplist = [list(a.ap[0])] + [[s, n] for (s, n) in dims]
        return AP(a.tensor, a.offset + base, aplist)

    with TileContext(nc) as tc:
        with (
            tc.tile_pool(name="io", bufs=2) as iop,
            tc.tile_pool(name="pp", bufs=2) as ppp,
        ):
            def dram_group_ap(dram, g, width):
                # [128p, T, width] view of rows [g*T*128, (g+1)*T*128):
                # row = g*T*128 + t*128 + p, iterated (p, t, f)
                a = dram[:]
                return AP(a.tensor, g * T * P_DIM * width,
                          [[width, P_DIM], [P_DIM * width, T], [1, width]])

            for g in range(n_groups * repeat):
                g = g % n_groups
                X1 = iop.tile([P_DIM, T * rep_dim], f32, tag="X1")
                X2 = iop.tile([P_DIM, T * rep_dim], f32, tag="X2")
                O = iop.tile([P_DIM, T * out_dim], f32, tag="O")
                # one big DMA per tensor: >=1MiB transfers split across all
                # 16 SDMA engines, and compute instructions then wait on at
                # most a couple of DMA semaphores (HW wait-slot limit).
                nc.gpsimd.dma_start(X1[:], dram_group_ap(x1d, g, rep_dim))
                nc.gpsimd.dma_start(X2[:], dram_group_ap(x2d, g, rep_dim))
                # wait absorbers: 4D-AP TensorTensor instructions cannot
                # carry embedded sync waits (S3S3D3 struct), so soak up the
                # DMA-complete waits with tiny 2D copies first.
                SCR = iop.tile([P_DIM, 4], f32, tag="SCR")
                nc.vector.tensor_copy(SCR[:, 0:2], X1[:, 0:2])
                nc.vector.tensor_copy(SCR[:, 2:4], X2[:, 0:2])
                if plan['needs_memset']:
                    nc.gpsimd.memset(O[:], 0.0)

                for _rep in range(compute_repeat):
                    for ci, csz in enumerate(plan['chunk_sizes']):
                        P = ppp.tile([P_DIM, T * csz], f32, tag="P")
                        for pi in plan['prod_instrs'][ci]:
                            dims = [(csz, T), (pi['ds'], pi['n']), (1, pi['d'])]
                            nc.vector.tensor_tensor(
                                ap_custom(P, pi['pslot'], dims),
                                ap_custom(X1, pi['a'],
                                          [(rep_dim, T), (pi['da'], pi['n']), (1, pi['d'])]),
                                ap_custom(X2, pi['b'],
                                          [(rep_dim, T), (pi['db'], pi['n']), (1, pi['d'])]),
                                mybir.AluOpType.mult,
                            )
                        for qi in plan['acc_instrs'][ci]:
                            w = qi['n'] * qi['d']   # collapsed contiguous width
                            o_ap = ap_custom(O, qi['o'], [(out_dim, T), (1, w)])
                            p_ap = ap_custom(P, qi['pslot'], [(csz, T), (1, w)])
                            if qi['kind'] == 'TS':
                                nc.vector.tensor_scalar_mul(o_ap, p_ap, float(qi['c']))
                            else:
                                nc.vector.scalar_tensor_tensor(
                                    out=o_ap, in0=p_ap, scalar=float(qi['c']),
                                    in1=o_ap,
                                    op0=mybir.AluOpType.mult,
                                    op1=mybir.AluOpType.add,
                                )

                nc.sync.dma_start(dram_group_ap(outd, g, out_dim), O[:])
    return nc


# ----------------------------------------------------------------------------
# Entry point
# ----------------------------------------------------------------------------

def kernel(x1, x2, cg_tilde, repids_in1, repids_in2, repids_out, out_dim):
    from concourse.bass_utils import run_bass_kernel_spmd

    x1 = np.asarray(x1, dtype=np.float32)
    x2 = np.asarray(x2, dtype=np.float32)
    cg = np.asarray(cg_tilde, dtype=np.float32)
    r1 = np.asarray(repids_in1).astype(np.int64)
    r2 = np.asarray(repids_in2).astype(np.int64)
    ro = np.asarray(repids_out).astype(np.int64)
    out_dim = int(out_dim)

    n, rep_dim = x1.shape
    rows_per_core = n // N_CORES

    key = (rows_per_core, rep_dim, out_dim, cg.tobytes(), r1.tobytes(),
           r2.tobytes(), ro.tobytes())
    cache_key = hash(key)
    if cache_key not in _BUILD_CACHE:
        plan = _build_plan(cg, r1, r2, ro, out_dim)
        nc = _build_bass(plan, rows_per_core, rep_dim, out_dim)
        _BUILD_CACHE[cache_key] = nc
    nc = _BUILD_CACHE[cache_key]

    in_maps = [
        {"x1": x1[i*rows_per_core:(i+1)*rows_per_core],
         "x2": x2[i*rows_per_core:(i+1)*rows_per_core]}
        for i in range(N_CORES)
    ]
    res = run_bass_kernel_spmd(nc, in_maps, list(range(N_CORES)))
    out = np.concatenate([res.results[i]["out"] for i in range(N_CORES)], axis=0)
    return out
